# revision 1
# baseline (speedup 1.0000x reference)
"""Multi-head attention TRN2 Bass kernel (8 NeuronCores, SPMD).

Problem: B=4, S=1024, E=1024, H=16 heads of dim 64, fp32.
    Q = q @ Wq^T (per head), K, V likewise
    scores = Q K^T / 8 ; P = softmax(scores) ; ctx = P V
    out = concat_heads(ctx) @ Wo^T

Sharding: core c handles batch b = c // 2 and head-group g = c % 2
(8 heads each). Each core computes a partial output projection over its
512 concat features; the host sums the two partials per batch (the
"unshard" for a reduction sharding).

Device design (no on-device transpose anywhere, all matmuls at the full
1 cycle/row bf16 PE rate; the whole kernel is one software pipeline):
  - Host passes x^T [E, S], per-head-transposed weight blocks
    wqT/wkT/wvT [E, 512] and woT [512, E], all pre-cast to bf16
    (matmul operands only; every accumulation stays fp32 in PSUM).
  - Inputs are host-pre-tiled into their exact SBUF-resident layout so
    each tensor loads with one flat 2D DMA (contiguous 8-16KB per
    partition, minimal descriptors), issued in consumption order.
  - Q^T, K^T produced in [d, s] layout with head pairs stacked to
    M=128; V in natural [t, d] layout, augmented with a ones block so
    the P@V matmul also emits the softmax denominator, broadcast across
    64 partitions (even heads [V|ones], odd heads [ones|V] so the
    denominator lands on the partitions the normalizing multiply needs).
  - scores^T [t, s]: K=64 row-tiled matmul pairs run CONCURRENTLY in
    disjoint PE row groups (measured 3ns apart), writing the two banks
    of one [128, 1024] PSUM tile; one ACT exp per pair-tile amortizes
    the ~190ns ACTIVATE overhead. The attention phase is exp-bound on
    the scalar engine, so the V projection and output projections are
    scheduled to fill the tensor engine's slack under it.
  - PSUM (8 banks): pp_big 2x[128,1024] rotates Q/K projection groups,
    score tiles and output-projection groups; pp_v 2x[128,512] keeps the
    V projection independent; pp_ctx 2x[128,512] holds the ctx/denom
    accumulators.
  - softmax without max-subtraction (scores ~N(0,1): exp is safe);
    normalization = fast-approx reciprocal (custom DVE op, base
    partition 0 only) + one DVE multiply per head on the tiny ctx^T,
    with a cross-partition SBUF->SBUF DMA for the reciprocal broadcast.
"""

from contextlib import ExitStack

import ml_dtypes
import numpy as np

import concourse.bacc as bacc
import concourse.mybir as mybir
import concourse.tile as tile
from concourse.bass_utils import run_bass_kernel_spmd

B, S, E, H = 4, 1024, 1024, 16
HD = 64          # head dim
HPC = 8          # heads per core
NPAIR = 4        # head pairs per core
NET = 8          # e-tiles (E / 128)
NTT = 8          # t-tiles (S / 128)
P = 128

F32 = mybir.dt.float32
BF16 = mybir.dt.bfloat16
EXP = mybir.ActivationFunctionType.Exp
SCALE = 1.0 / 8.0  # 1/sqrt(HD)
BF = ml_dtypes.bfloat16


def _emit(nc, tc, ctx, aps):
    xqT, xkT, xvT, wqT, wkT, wvT, woT, out = aps

    xpool = ctx.enter_context(tc.tile_pool(name="xpool", bufs=3))
    wpool = ctx.enter_context(tc.tile_pool(name="wpool", bufs=3))
    const = ctx.enter_context(tc.tile_pool(name="const", bufs=1))
    etp = ctx.enter_context(tc.tile_pool(name="etp", bufs=16))
    obp = ctx.enter_context(tc.tile_pool(name="obp", bufs=3))
    rcp = ctx.enter_context(tc.tile_pool(name="rcp", bufs=8))
    pp_mm = ctx.enter_context(tc.tile_pool(name="pp_mm", bufs=2, space="PSUM"))
    pp_sc = ctx.enter_context(tc.tile_pool(name="pp_sc", bufs=2, space="PSUM"))
    pp_ctx = ctx.enter_context(tc.tile_pool(name="pp_ctx", bufs=2, space="PSUM"))

    wo_t = const.tile([P, 4096], BF16, name="wo_t")
    qt = const.tile([P, 4096], BF16, name="qt")
    kt = const.tile([P, 4096], BF16, name="kt")
    vaug = const.tile([P, 8192], BF16, name="vaug")
    cat = const.tile([P, 4096], BF16, name="cat")

    # ones blocks of the V augmentation (see module docstring)
    v4 = vaug[:, :].rearrange("p (j q c) -> p j q c", q=2, c=P)
    nc.gpsimd.memset(v4[:, :, 0, HD:P], 1.0)
    nc.gpsimd.memset(v4[:, :, 1, 0:HD], 1.0)

    def load_wx(wT, xT):
        w = wpool.tile([P, NET * 512], BF16, name="w", tag="wt")
        nc.sync.dma_start(out=w[:], in_=wT[:])
        x = xpool.tile([P, NET * 1024], BF16, name="x", tag="xt")
        half = NET * 512
        nc.sync.dma_start(out=x[:, 0:half], in_=xT[:, 0:half])
        nc.sync.dma_start(out=x[:, half:2 * half], in_=xT[:, half:2 * half])
        return w, x

    wq, xq = load_wx(wqT, xqT)
    wk, xk = load_wx(wkT, xkT)
    wv, xv = load_wx(wvT, xvT)
    nc.sync.dma_start(out=wo_t[:], in_=woT[:])

    # ---- Q/K projections: both s-halves interleave in one 2-bank tile,
    # so consecutive matmuls share each weight load and one [128,1024]
    # copy drains the pair. Q and K alternate per head pair so the
    # attention of pair 0 (and with it the critical exp stream on the
    # scalar engine) can start as soon as possible. ----
    def proj_pair(w, x, dst, p):
        for sh in range(2):
            ps = pp_mm.tile([P, 512], F32, name="ps", tag="mm")
            for et in range(NET):
                nc.tensor.matmul(
                    ps[:],
                    lhsT=w[:, et * 512 + p * P:et * 512 + (p + 1) * P],
                    rhs=x[:, et * 1024 + sh * 512:et * 1024 + (sh + 1) * 512],
                    start=(et == 0), stop=(et == NET - 1),
                )
            nc.vector.tensor_copy(
                dst[:, p * 1024 + sh * 512:p * 1024 + (sh + 1) * 512], ps[:])



    # ---- attention (emitted before the V projection: the exp stream on
    # the scalar engine is the phase's critical path and must start as
    # early as possible; V-projection matmuls fill PE slack under it and
    # the ctx matmuls wait on their vaug blocks via Tile deps) ----
    def normalize_a(ctx_ps, qcol):
        # ctx rows 0:64, denominator rows 64:128. reciprocal_approx_fast
        # only works at base partition 0: move the denominator down first.
        rA = rcp.tile([P, 512], F32, name="rA", tag="rc")
        rA2 = rcp.tile([P, 512], F32, name="rA2", tag="rc")
        nc.vector.tensor_copy(rA[HD:P, :], ctx_ps[HD:P, :])
        nc.sync.dma_start(out=rA[0:HD, :], in_=rA[HD:P, :])
        nc.vector.reciprocal_approx_fast(rA2[0:HD, :], rA[0:HD, :])
        nc.vector.tensor_mul(cat[0:HD, qcol:qcol + 512],
                             ctx_ps[0:HD, :], rA2[0:HD, :])

    def normalize_b(ctx_ps, qcol):
        # mirrored: denominator rows 0:64, ctx rows 64:128
        rB = rcp.tile([P, 512], F32, name="rB", tag="rc")
        nc.vector.reciprocal_approx_fast(rB[0:HD, :], ctx_ps[0:HD, :])
        nc.sync.dma_start(out=rB[HD:P, :], in_=rB[0:HD, :])
        nc.vector.tensor_mul(cat[HD:P, qcol:qcol + 512],
                             ctx_ps[HD:P, :], rB[HD:P, :])

    def attention_pair(sh, p):
            qcol = p * 1024 + sh * 512
            ctxA = pp_ctx.tile([P, 512], F32, name="ctxA", tag="ctx")
            ctxB = pp_ctx.tile([P, 512], F32, name="ctxB", tag="ctx")
            for tt in range(NTT):
                kcol = p * 1024 + tt * P
                sAB = pp_sc.tile([P, 1024], F32, name="sAB", tag="sc")
                nc.tensor.matmul(
                    sAB[:, 0:512],
                    lhsT=kt[0:HD, kcol:kcol + P],
                    rhs=qt[0:HD, qcol:qcol + 512],
                    start=True, stop=True)
                nc.tensor.matmul(
                    sAB[:, 512:1024],
                    lhsT=kt[HD:P, kcol:kcol + P],
                    rhs=qt[HD:P, qcol:qcol + 512],
                    start=True, stop=True)
                eAB = etp.tile([P, 1024], BF16, name="eAB", tag="et")
                nc.scalar.activation(eAB[:], sAB[:], EXP, scale=SCALE)
                bA = (tt * HPC + 2 * p) * P
                bB = bA + P
                nc.tensor.matmul(ctxA[:], lhsT=vaug[:, bA:bA + P],
                                 rhs=eAB[:, 0:512],
                                 start=(tt == 0), stop=(tt == NTT - 1))
                nc.tensor.matmul(ctxB[:], lhsT=vaug[:, bB:bB + P],
                                 rhs=eAB[:, 512:1024],
                                 start=(tt == 0), stop=(tt == NTT - 1))
            normalize_a(ctxA, qcol)
            normalize_b(ctxB, qcol)

    def outproj(sh):
        # partial over our 512 concat features. The first half runs on
        # the pp_mm rotation (it overlaps the still-running attention);
        # the last half runs on the score banks, which are free by then,
        # with both i-halves interleaved per 2-bank tile so the final
        # tail streams at full rate.
        if sh == 0:
            for j in range(4):
                st = sh * 4 + j
                for ih in range(2):
                    ps = pp_mm.tile([P, 512], F32, name="po", tag="mm")
                    for p4 in range(4):
                        nc.tensor.matmul(
                            ps[:],
                            lhsT=cat[:, p4 * 1024 + st * P:p4 * 1024 + (st + 1) * P],
                            rhs=wo_t[:, p4 * 1024 + ih * 512:p4 * 1024 + (ih + 1) * 512],
                            start=(p4 == 0), stop=(p4 == 3))
                    ob = obp.tile([P, 512], F32, name="ob", tag="ob")
                    nc.vector.tensor_copy(ob[:], ps[:])
                    nc.sync.dma_start(
                        out=out[st * P:(st + 1) * P, ih * 512:(ih + 1) * 512],
                        in_=ob[:])
        else:
            for j in range(4):
                st = sh * 4 + j
                ps = pp_sc.tile([P, 1024], F32, name="po2", tag="sc")
                # rotate the accumulation order so the in-flight groups
                # need the last head pair only for their final matmul
                for k4 in range(4):
                    p4 = (k4 + j) % 4 if j < 2 else k4
                    lhsT = cat[:, p4 * 1024 + st * P:p4 * 1024 + (st + 1) * P]
                    for ih in range(2):
                        nc.tensor.matmul(
                            ps[:, ih * 512:(ih + 1) * 512],
                            lhsT=lhsT,
                            rhs=wo_t[:, p4 * 1024 + ih * 512:p4 * 1024 + (ih + 1) * 512],
                            start=(k4 == 0), stop=(k4 == 3))
                ob = obp.tile([P, 1024], F32, name="ob2", tag="ob2")
                nc.vector.tensor_copy(ob[:], ps[:])
                nc.sync.dma_start(out=out[st * P:(st + 1) * P, :], in_=ob[:])

    # ---- V projection: natural [t, hd] layout into vaug blocks ----
    def vproj():
      for tt in range(NTT):
        ps = pp_mm.tile([P, 512], F32, name="psv", tag="mm")
        for et in range(NET):
            nc.tensor.matmul(
                ps[:],
                lhsT=xv[:, et * 1024 + tt * P:et * 1024 + (tt + 1) * P],
                rhs=wv[:, et * 512:(et + 1) * 512],
                start=(et == 0), stop=(et == NET - 1),
            )
        # psum cols h*64+d ; even heads -> block cols 0:64, odd -> 64:128
        dstt = vaug[:, tt * 1024:(tt + 1) * 1024].rearrange(
            "p (j q c) -> p j q c", q=2, c=P)
        srcv = ps[:].rearrange("p (j q c) -> p j q c", q=2, c=HD)
        nc.vector.tensor_copy(dstt[:, :, 0, 0:HD], srcv[:, :, 0, :])
        nc.vector.tensor_copy(dstt[:, :, 1, HD:P], srcv[:, :, 1, :])

    # Q/K projections interleave with the attention per head pair: pair
    # p's scores (both s-halves) depend only on pair p's projections, so
    # the exp stream starts right after pair 0 and stays fed while the
    # remaining projections and the V projection fill the PE. (Tile-pool
    # slots are granted in declaration order, which makes this emission
    # order the schedule.) The first output projection slots in before
    # the very last attention block to overlap its exp tail.
    proj_pair(wq, xq, qt, 0)
    proj_pair(wk, xk, kt, 0)
    vproj()
    for p in range(NPAIR):
        if p > 0:
            proj_pair(wq, xq, qt, p)
            proj_pair(wk, xk, kt, p)
        attention_pair(0, p)
        if p == NPAIR - 1:
            outproj(0)
        attention_pair(1, p)
    outproj(1)


_CACHE = {}


def build():
    if "nc" in _CACHE:
        return _CACHE["nc"]
    nc = bacc.Bacc("TRN2", target_bir_lowering=False, debug=False)
    xqT = nc.dram_tensor("xqT", [P, NET * S], BF16, kind="ExternalInput").ap()
    xkT = nc.dram_tensor("xkT", [P, NET * S], BF16, kind="ExternalInput").ap()
    xvT = nc.dram_tensor("xvT", [P, NET * S], BF16, kind="ExternalInput").ap()
    wqT = nc.dram_tensor("wqT", [P, NET * HPC * HD], BF16, kind="ExternalInput").ap()
    wkT = nc.dram_tensor("wkT", [P, NET * HPC * HD], BF16, kind="ExternalInput").ap()
    wvT = nc.dram_tensor("wvT", [P, NET * HPC * HD], BF16, kind="ExternalInput").ap()
    woT = nc.dram_tensor("woT", [P, 4 * E], BF16, kind="ExternalInput").ap()
    out = nc.dram_tensor("out", [S, E], F32, kind="ExternalOutput").ap()
    with tile.TileContext(nc) as tc, ExitStack() as ctx:
        _emit(nc, tc, ctx, (xqT, xkT, xvT, wqT, wkT, wvT, woT, out))
    nc.compile()
    _CACHE["nc"] = nc
    return nc


def make_in_maps(query, key, value, Wq, Wk, Wv, Wo):
    in_maps = []
    for c in range(8):
        b, g = divmod(c, 2)
        hs = slice(g * HPC, (g + 1) * HPC)

        def bf(a):
            return np.ascontiguousarray(a).astype(BF)

        def sbuf_tile(a):
            # [E_or_512, N] -> the SBUF-resident layout [128, n_et * N]:
            # row p, col et*N+c  =  a[et*128 + p, c]
            et = a.shape[0] // P
            return bf(a.reshape(et, P, -1).transpose(1, 0, 2).reshape(P, -1))

        # x^T [E, S]; w blocks [E, 512] with col h*64+d = W[g*8+h, d, e];
        # woT [512, E] with woT[hd, i] = Wo[i, g*512+hd]
        in_maps.append({
            "xqT": sbuf_tile(np.asarray(query[b], np.float32).T),
            "xkT": sbuf_tile(np.asarray(key[b], np.float32).T),
            "xvT": sbuf_tile(np.asarray(value[b], np.float32).T),
            "wqT": sbuf_tile(np.asarray(Wq[hs], np.float32).transpose(2, 0, 1).reshape(E, HPC * HD)),
            "wkT": sbuf_tile(np.asarray(Wk[hs], np.float32).transpose(2, 0, 1).reshape(E, HPC * HD)),
            "wvT": sbuf_tile(np.asarray(Wv[hs], np.float32).transpose(2, 0, 1).reshape(E, HPC * HD)),
            "woT": sbuf_tile(np.asarray(Wo[:, g * HPC * HD:(g + 1) * HPC * HD], np.float32).T),
        })
    return in_maps


def kernel(query, key, value, Wq, Wk, Wv, Wo):
    nc = build()
    in_maps = make_in_maps(query, key, value, Wq, Wk, Wv, Wo)
    res = run_bass_kernel_spmd(nc, in_maps, list(range(8))).results
    out = np.empty((B, S, E), np.float32)
    for b in range(B):
        out[b] = res[2 * b]["out"] + res[2 * b + 1]["out"]
    return out



# revision 7
# speedup vs baseline: 1.1552x; 1.1552x over previous
"""Multi-head attention TRN2 Bass kernel (8 NeuronCores, SPMD), v2.

Problem: B=4, S=1024, E=1024, H=16 heads of dim 64, fp32.
Sharding: core c = (batch c//2, head-group c%2); host sums the two
partial output projections per batch.

v2 schedule (vs baseline): the scalar-engine exp stream (64 ACTIVATEs,
~96us with sem overhead) is the co-critical path with the PE (448
512-col matmul units, ~96us at 2.4GHz). The emitter paces score tiles
so exp starts ~+5us (baseline +32us) and never starves; Q/K/V/output
projections are emitted as PE filler between score tiles, ordered by
DMA arrival. ctx matmuls lag the exp stream via a deep e-tile pool.
Inputs land as many small chunked DMAs ordered by first use.
"""

from contextlib import ExitStack

import ml_dtypes
import numpy as np

import concourse.bacc as bacc
import concourse.mybir as mybir
import concourse.tile as tile
from concourse.bass_utils import run_bass_kernel_spmd

B, S, E, H = 4, 1024, 1024, 16
HD = 64
HPC = 8
NPAIR = 4
NET = 8
NTT = 8
P = 128

F32 = mybir.dt.float32
BF16 = mybir.dt.bfloat16
EXP = mybir.ActivationFunctionType.Exp
SCALE = 1.0 / 8.0
BF = ml_dtypes.bfloat16

# --- emission-time pacing model (ns); only shapes the static order ---
U = 216.0          # one 512-col bf16 matmul @2.4GHz
ACT_NS = 1500.0    # one [128,1024] exp incl sem overhead
DRAIN_NS = 800.0   # DVE psum->sbuf drain
NORM_NS = 2600.0   # normalize chain (copy+dma+recip+mul)
DMA_BW = 0.360     # bytes/ns aggregate
GUARD = 350.0
LAG = 3            # ctx tiles lag behind exp tiles


def _emit(nc, tc, ctx, aps):
    (wq_d, wk_d, wv_d, xq_d, xk_d, xv_d, wo_d, out_d) = aps

    iw = ctx.enter_context(tc.tile_pool(name="iw", bufs=1))
    ix = ctx.enter_context(tc.tile_pool(name="ix", bufs=1))
    const = ctx.enter_context(tc.tile_pool(name="const", bufs=1))
    ETP_BUFS = 16
    etp = ctx.enter_context(tc.tile_pool(name="etp", bufs=ETP_BUFS))
    obp = ctx.enter_context(tc.tile_pool(name="obp", bufs=3))
    rcp = ctx.enter_context(tc.tile_pool(name="rcp", bufs=4))
    pp_sc = ctx.enter_context(tc.tile_pool(name="pp_sc", bufs=2, space="PSUM"))
    pp_ctx = ctx.enter_context(tc.tile_pool(name="pp_ctx", bufs=2, space="PSUM"))
    pp_mm = ctx.enter_context(tc.tile_pool(name="pp_mm", bufs=2, space="PSUM"))

    qt = const.tile([P, 4096], BF16, name="qt")
    kt = const.tile([P, 4096], BF16, name="kt")
    vaug = const.tile([P, 8192], BF16, name="vaug")
    cat = const.tile([P, 4096], BF16, name="cat")
    wo_t = const.tile([P, 4096], BF16, name="wo_t")
    warm = const.tile([P, 16], F32, name="warm")

    # warm the ACT exp table during the DMA prologue (no data deps)
    nc.scalar.activation(warm[:, 0:8], warm[:, 8:16], EXP, scale=1.0)

    # ones blocks of the V augmentation
    v4 = vaug[:, :].rearrange("p (j q c) -> p j q c", q=2, c=P)
    nc.gpsimd.memset(v4[:, :, 0, HD:P], 1.0)
    nc.gpsimd.memset(v4[:, :, 1, 0:HD], 1.0)

    # ---- input tiles ----
    wqp = [iw.tile([P, 1024], BF16, name=f"wqp{p}") for p in range(NPAIR)]
    wkp = [iw.tile([P, 1024], BF16, name=f"wkp{p}") for p in range(NPAIR)]
    wvt = iw.tile([P, 4096], BF16, name="wvt")
    xqt = [ix.tile([P, 4096], BF16, name=f"xqt{h}") for h in range(2)]
    xkt = [ix.tile([P, 4096], BF16, name=f"xkt{h}") for h in range(2)]
    xvc = [ix.tile([P, 2048], BF16, name=f"xvc{c}") for c in range(4)]

    # ---- DMA plan: (sbuf_tile, dram_ap) in issue order = arrival order ----
    dma_plan = [
        ("wk0", wkp[0], wk_d[:, 0:1024]),
        ("xk0", xkt[0], xk_d[:, 0:4096]),
        ("wq0", wqp[0], wq_d[:, 0:1024]),
        ("xq0", xqt[0], xq_d[:, 0:4096]),
        ("xk1", xkt[1], xk_d[:, 4096:8192]),
        ("wv", wvt, wv_d[:, :]),
        ("xv0", xvc[0], xv_d[:, 0:2048]),
        ("xv1", xvc[1], xv_d[:, 2048:4096]),
        ("wk1", wkp[1], wk_d[:, 1024:2048]),
        ("wq1", wqp[1], wq_d[:, 1024:2048]),
        ("xv2", xvc[2], xv_d[:, 4096:6144]),
        ("xv3", xvc[3], xv_d[:, 6144:8192]),
        ("wk2", wkp[2], wk_d[:, 2048:3072]),
        ("wq2", wqp[2], wq_d[:, 2048:3072]),
        ("xq1", xqt[1], xq_d[:, 4096:8192]),
        ("wk3", wkp[3], wk_d[:, 3072:4096]),
        ("wq3", wqp[3], wq_d[:, 3072:4096]),
        ("wo", wo_t, wo_d[:, :]),
    ]
    ready = {}
    cum = 0.0
    for idx, (nm, t, src) in enumerate(dma_plan):
        nc.sync.dma_start(out=t[:], in_=src)
        nbytes = t.shape[0] * t.shape[1] * 2
        cum += nbytes
        ready[nm] = cum / DMA_BW + idx * 600.0 + 1200.0

    # ---- emission-time state ----
    st = {"pe": 4000.0, "act": 0.0}

    def pe_run(units, dep_ns=0.0):
        st["pe"] = max(st["pe"], dep_ns) + units * U
        return st["pe"]

    # ---- op emitters ----
    proj_done = {}   # ("q"|"k", p, h) -> drain-done est

    def qk_proj(which, p, h):
        if (which, p, h) in proj_done:
            return
        w = wqp[p] if which == "q" else wkp[p]
        x = xqt[h] if which == "q" else xkt[h]
        dst = qt if which == "q" else kt
        dep = max(ready[f"w{which}{p}"], ready[f"x{which}{h}"])
        ps = pp_mm.tile([P, 512], F32, name="ps", tag="mm")
        for et in range(NET):
            nc.tensor.matmul(
                ps[:],
                lhsT=w[:, et * P:(et + 1) * P],
                rhs=x[:, et * 512:(et + 1) * 512],
                start=(et == 0), stop=(et == NET - 1),
            )
        nc.vector.tensor_copy(
            dst[:, p * 1024 + h * 512:p * 1024 + (h + 1) * 512], ps[:])
        t = pe_run(8, dep)
        proj_done[(which, p, h)] = t + DRAIN_NS

    vproj_done = {}

    def vproj(tt):
        if tt in vproj_done:
            return
        dep = max(ready["wv"], ready[f"xv{tt // 2}"])
        x = xvc[tt // 2]
        base = (tt % 2) * 1024
        ps = pp_mm.tile([P, 512], F32, name="psv", tag="mm")
        for et in range(NET):
            nc.tensor.matmul(
                ps[:],
                lhsT=x[:, base + et * P:base + (et + 1) * P],
                rhs=wvt[:, et * 512:(et + 1) * 512],
                start=(et == 0), stop=(et == NET - 1),
            )
        dstt = vaug[:, tt * 1024:(tt + 1) * 1024].rearrange(
            "p (j q c) -> p j q c", q=2, c=P)
        srcv = ps[:].rearrange("p (j q c) -> p j q c", q=2, c=HD)
        nc.vector.tensor_copy(dstt[:, :, 0, 0:HD], srcv[:, :, 0, :])
        nc.vector.tensor_copy(dstt[:, :, 1, HD:P], srcv[:, :, 1, :])
        t = pe_run(8, dep)
        vproj_done[tt] = t + DRAIN_NS

    # attention call order: sh-major. call = sh*4 + p
    CALLS = [(p, 0) for p in range(NPAIR)] + [(p, 1) for p in range(NPAIR)]

    e_tiles = {}     # (call, tt) -> sbuf e tile

    def sc_exp(call, tt):
        p, sh = CALLS[call]
        qcol = p * 1024 + sh * 512
        kcol = p * 1024 + tt * P
        sAB = pp_sc.tile([P, 1024], F32, name="sAB", tag="sc")
        nc.tensor.matmul(sAB[:, 0:512], lhsT=kt[0:HD, kcol:kcol + P],
                         rhs=qt[0:HD, qcol:qcol + 512], start=True, stop=True)
        nc.tensor.matmul(sAB[:, 512:1024], lhsT=kt[HD:P, kcol:kcol + P],
                         rhs=qt[HD:P, qcol:qcol + 512], start=True, stop=True)
        eAB = etp.tile([P, 1024], BF16, name="eAB", tag="et")
        nc.scalar.activation(eAB[:], sAB[:], EXP, scale=SCALE)
        e_tiles[(call, tt)] = eAB
        t = pe_run(1)
        st["act"] = max(st["act"] + 170.0, t + 300.0) + ACT_NS

    ctx_ps = {}      # call -> (ctxA, ctxB)
    norm_done = {}

    def ctx_mm(call, tt):
        p, sh = CALLS[call]
        if tt == 0:
            ctx_ps[call] = (
                pp_ctx.tile([P, 512], F32, name="ctxA", tag="ctx"),
                pp_ctx.tile([P, 512], F32, name="ctxB", tag="ctx"),
            )
        ctxA, ctxB = ctx_ps[call]
        eAB = e_tiles.pop((call, tt))
        bA = tt * 1024 + p * 256
        nc.tensor.matmul(ctxA[:], lhsT=vaug[:, bA:bA + P],
                         rhs=eAB[:, 0:512],
                         start=(tt == 0), stop=(tt == NTT - 1))
        nc.tensor.matmul(ctxB[:], lhsT=vaug[:, bA + P:bA + 2 * P],
                         rhs=eAB[:, 512:1024],
                         start=(tt == 0), stop=(tt == NTT - 1))
        pe_run(2)
        if tt == NTT - 1:
            normalize(call)

    def normalize(call):
        p, sh = CALLS[call]
        qcol = p * 1024 + sh * 512
        ctxA, ctxB = ctx_ps.pop(call)
        # A: ctx rows 0:64, denom rows 64:128 (64 identical copies)
        rA = rcp.tile([P, 512], F32, name="rA", tag="rc")
        rA2 = rcp.tile([P, 512], F32, name="rA2", tag="rc")
        nc.vector.tensor_copy(rA[HD:P, :], ctxA[HD:P, :])
        nc.sync.dma_start(out=rA[0:HD, :], in_=rA[HD:P, :])
        nc.vector.reciprocal_approx_fast(rA2[0:HD, :], rA[0:HD, :])
        nc.vector.tensor_mul(cat[0:HD, qcol:qcol + 512],
                             ctxA[0:HD, :], rA2[0:HD, :])
        # B: denom rows 0:64, ctx rows 64:128
        rB = rcp.tile([P, 512], F32, name="rB", tag="rc")
        nc.vector.reciprocal_approx_fast(rB[0:HD, :], ctxB[0:HD, :])
        nc.sync.dma_start(out=rB[HD:P, :], in_=rB[0:HD, :])
        nc.vector.tensor_mul(cat[HD:P, qcol:qcol + 512],
                             ctxB[HD:P, :], rB[HD:P, :])
        norm_done[call] = st["pe"] + NORM_NS

    def outgrp(sti, ih):
        # partial product for out rows st*128.. over our 512 concat features
        ps = pp_mm.tile([P, 512], F32, name="po", tag="mm")
        for p4 in range(NPAIR):
            nc.tensor.matmul(
                ps[:],
                lhsT=cat[:, p4 * 1024 + sti * P:p4 * 1024 + (sti + 1) * P],
                rhs=wo_t[:, p4 * 1024 + ih * 512:p4 * 1024 + (ih + 1) * 512],
                start=(p4 == 0), stop=(p4 == NPAIR - 1))
        ob = obp.tile([P, 512], F32, name="ob", tag="ob")
        nc.vector.tensor_copy(ob[:], ps[:])
        nc.sync.dma_start(
            out=out_d[sti * P:(sti + 1) * P, ih * 512:(ih + 1) * 512],
            in_=ob[:])
        pe_run(4)

    # ---- filler queue: (dep_est_fn, emit_fn, units) ----
    def F(dep, fn):
        return {"dep": dep, "fn": fn}

    fillers = [
        F(lambda: max(ready["wk0"], ready["xk1"]), lambda: qk_proj("k", 0, 1)),
        F(lambda: max(ready["wv"], ready["xv0"]), lambda: vproj(0)),
        F(lambda: max(ready["wv"], ready["xv0"]), lambda: vproj(1)),
        F(lambda: max(ready["wv"], ready["xv1"]), lambda: vproj(2)),
        F(lambda: max(ready["wv"], ready["xv1"]), lambda: vproj(3)),
        F(lambda: max(ready["wk1"], ready["xk0"]), lambda: qk_proj("k", 1, 0)),
        F(lambda: max(ready["wq1"], ready["xq0"]), lambda: qk_proj("q", 1, 0)),
        F(lambda: max(ready["wk1"], ready["xk1"]), lambda: qk_proj("k", 1, 1)),
        F(lambda: max(ready["wv"], ready["xv2"]), lambda: vproj(4)),
        F(lambda: max(ready["wv"], ready["xv2"]), lambda: vproj(5)),
        F(lambda: max(ready["wv"], ready["xv3"]), lambda: vproj(6)),
        F(lambda: max(ready["wv"], ready["xv3"]), lambda: vproj(7)),
        F(lambda: max(ready["wk2"], ready["xk0"]), lambda: qk_proj("k", 2, 0)),
        F(lambda: max(ready["wq2"], ready["xq0"]), lambda: qk_proj("q", 2, 0)),
        F(lambda: max(ready["wk2"], ready["xk1"]), lambda: qk_proj("k", 2, 1)),
        F(lambda: max(ready["wk3"], ready["xk0"]), lambda: qk_proj("k", 3, 0)),
        F(lambda: max(ready["wq3"], ready["xq0"]), lambda: qk_proj("q", 3, 0)),
        F(lambda: max(ready["wk3"], ready["xk1"]), lambda: qk_proj("k", 3, 1)),
        F(lambda: max(ready["wq0"], ready["xq1"]), lambda: qk_proj("q", 0, 1)),
        F(lambda: max(ready["wq1"], ready["xq1"]), lambda: qk_proj("q", 1, 1)),
        F(lambda: max(ready["wq2"], ready["xq1"]), lambda: qk_proj("q", 2, 1)),
        F(lambda: max(ready["wq3"], ready["xq1"]), lambda: qk_proj("q", 3, 1)),
    ]
    # sh0 output projections become available mid-kernel (after call 3)
    for sti in range(4):
        for ih in range(2):
            fillers.append(F(
                lambda: max(norm_done.get(3, 1e18), ready["wo"]),
                lambda sti=sti, ih=ih: outgrp(sti, ih)))

    fill_i = [0]

    def try_filler():
        i = fill_i[0]
        if i >= len(fillers):
            return False
        if fillers[i]["dep"]() <= st["pe"] + 150.0:
            fillers[i]["fn"]()
            fill_i[0] += 1
            return True
        return False

    def force_filler():
        i = fill_i[0]
        assert i < len(fillers)
        fillers[i]["fn"]()
        fill_i[0] += 1

    # ---- main schedule ----
    qk_proj("k", 0, 0)
    qk_proj("q", 0, 0)

    ctx_n = [0]   # next ctx tile index (flat: call*8 + tt)

    def ctx_emittable(sc_emitted):
        k = ctx_n[0]
        if k >= 64 or k > sc_emitted - LAG:
            return False
        call, tt = divmod(k, NTT)
        if vproj_done.get(tt, 1e18) > st["pe"] + 100.0:
            return False
        if tt == 0 and call > 0 and norm_done.get(call - 1, 1e18) > st["pe"]:
            return False
        return True

    for i in range(64):
        call, tt = divmod(i, NTT)
        p, sh = CALLS[call]
        # e-pool safety: ctx(k) must be emitted before sc_exp(k + ETP_BUFS)
        while ctx_n[0] <= i - ETP_BUFS:
            ck, ctt = divmod(ctx_n[0], NTT)
            vproj(ctt)
            ctx_mm(ck, ctt)
            ctx_n[0] += 1
        # make sure the projections this score tile needs are emitted
        need = [("q", p, sh), ("k", p, 0)] + ([("k", p, 1)] if tt >= 4 else [])
        for nd in need:
            if nd not in proj_done:
                qk_proj(*nd)
        sc_exp(call, tt)
        # catch-up work until PE pace matches ACT pace
        while st["pe"] < st["act"] - GUARD:
            if ctx_emittable(i + 1):
                ctx_mm(*divmod(ctx_n[0], NTT))
                ctx_n[0] += 1
            elif try_filler():
                pass
            else:
                # if ctx is blocked only on an unemitted vproj, force fillers
                k = ctx_n[0]
                if (k < 64 and k <= i + 1 - LAG
                        and fill_i[0] < len(fillers)):
                    force_filler()
                else:
                    break

    # drain remaining ctx tiles + any leftover fillers
    while ctx_n[0] < 64:
        if ctx_emittable(64 + LAG):
            ctx_mm(*divmod(ctx_n[0], NTT))
            ctx_n[0] += 1
        elif fill_i[0] < len(fillers):
            force_filler()
        else:
            # ctx blocked on emission-order estimate only; emit anyway
            ctx_mm(*divmod(ctx_n[0], NTT))
            ctx_n[0] += 1
    while fill_i[0] < len(fillers):
        force_filler()

    # tail: sh1 output projections (st 4..7)
    for sti in range(4, 8):
        for ih in range(2):
            outgrp(sti, ih)


_CACHE = {}


def build():
    if "nc" in _CACHE:
        return _CACHE["nc"]
    nc = bacc.Bacc("TRN2", target_bir_lowering=False, debug=False)
    wq_d = nc.dram_tensor("wq", [P, 4096], BF16, kind="ExternalInput").ap()
    wk_d = nc.dram_tensor("wk", [P, 4096], BF16, kind="ExternalInput").ap()
    wv_d = nc.dram_tensor("wv", [P, 4096], BF16, kind="ExternalInput").ap()
    xq_d = nc.dram_tensor("xq", [P, 8192], BF16, kind="ExternalInput").ap()
    xk_d = nc.dram_tensor("xk", [P, 8192], BF16, kind="ExternalInput").ap()
    xv_d = nc.dram_tensor("xv", [P, 8192], BF16, kind="ExternalInput").ap()
    wo_d = nc.dram_tensor("wo", [P, 4096], BF16, kind="ExternalInput").ap()
    out_d = nc.dram_tensor("out", [S, E], F32, kind="ExternalOutput").ap()
    with tile.TileContext(nc) as tc, ExitStack() as ctx:
        _emit(nc, tc, ctx, (wq_d, wk_d, wv_d, xq_d, xk_d, xv_d, wo_d, out_d))
    nc.compile()
    _CACHE["nc"] = nc
    return nc


def make_in_maps(query, key, value, Wq, Wk, Wv, Wo):
    in_maps = []
    for c in range(8):
        b, g = divmod(c, 2)
        hs = slice(g * HPC, (g + 1) * HPC)

        def bf(a):
            return np.ascontiguousarray(a, dtype=np.float32).astype(BF)

        # per-pair Q/K weights: w[ep, p*1024 + et*128 + hp*64 + d]
        def wqk(Wfull):
            A = np.asarray(Wfull[hs], np.float32)          # [8, 64, 1024]
            Ap = A.reshape(NPAIR, 2, HD, NET, P)           # [p, hp, d, et, ep]
            Ap = Ap.transpose(4, 0, 3, 1, 2)               # [ep, p, et, hp, d]
            return bf(Ap.reshape(P, 4096))

        # x for Q/K: x[ep, h*4096 + et*512 + s]
        def xqk(x):
            X = np.asarray(x, np.float32)                  # [1024 s, 1024 e]
            Xp = X.reshape(2, 512, NET, P)                 # [h, s, et, ep]
            Xp = Xp.transpose(3, 0, 2, 1)                  # [ep, h, et, s]
            return bf(Xp.reshape(P, 8192))

        # x for V: x[ep, tt*1024 + et*128 + tl]
        V = np.asarray(value[b], np.float32)
        Vp = V.reshape(NTT, P, NET, P).transpose(3, 0, 2, 1)  # [ep, tt, et, tl]
        xv = bf(Vp.reshape(P, 8192))

        # wv: [ep, et*512 + h*64 + d]
        Bv = np.asarray(Wv[hs], np.float32)                # [8 h, 64 d, 1024 e]
        Bp = Bv.reshape(HPC, HD, NET, P).transpose(3, 2, 0, 1)
        wv = bf(Bp.reshape(P, 4096))

        # wo: [f, p4*1024 + i]
        C = np.asarray(Wo[:, g * 512:(g + 1) * 512], np.float32)  # [1024 i, 512 f]
        Cp = C.reshape(E, NPAIR, P).transpose(2, 1, 0)     # [f, p4, i]
        wo = bf(Cp.reshape(P, 4096))

        in_maps.append({
            "wq": wqk(Wq), "wk": wqk(Wk), "wv": wv,
            "xq": xqk(query[b]), "xk": xqk(key[b]), "xv": xv,
            "wo": wo,
        })
    return in_maps


def kernel(query, key, value, Wq, Wk, Wv, Wo):
    nc = build()
    in_maps = make_in_maps(query, key, value, Wq, Wk, Wv, Wo)
    res = run_bass_kernel_spmd(nc, in_maps, list(range(8))).results
    out = np.empty((B, S, E), np.float32)
    for b in range(B):
        out[b] = res[2 * b]["out"] + res[2 * b + 1]["out"]
    return out


# revision 11
# speedup vs baseline: 1.2448x; 1.0776x over previous
"""Multi-head attention TRN2 Bass kernel (8 NeuronCores, SPMD), v3.

Problem: B=4, S=1024, E=1024, H=16 heads of dim 64, fp32.
Sharding: core c = (batch c//2, head-group c%2); host sums the two
partial output projections per batch.

The kernel is PE-bound: 448 512-col bf16 matmul units (~121us at the
sustained ~2.0GHz P0 clock), with the scalar-engine exp stream (64
ACTIVATEs, ~80-96us) as co-critical path. v3 scheduling:
  - inputs split across BOTH HWDGE rings (SP + ACT) in 0.25-0.5MB
    chunks ordered by first use, so the first score tile's inputs
    (wk0/xk0 on SP, wq0/xq0 on ACT) land right after the queue preamble
  - one global in-order PE schedule: score-tile stream (1u each) with
    ctx accumulation lagging LAG tiles behind the exp stream, and
    projection/output-projection filler paced by a PE-work budget
    (~2.9u filler per score tile) in 4-matmul chunks
  - normalize copies ctx PSUM to SBUF first, freeing the 2 ctx banks
    immediately so consecutive (pair,sh) calls never stall the PE
  - output projection: st0-3 stream as filler during sh1 attention;
    st4-7 pre-accumulate pairs 0-2 after the last score tile (reusing
    the freed score/ctx/proj PSUM banks) and finish with pair 3 right
    after the final normalize, keeping the tail short
  - qt/kt/vaug/cat are split into per-producer tiles so conservative
    dependency tracking cannot serialize unrelated stages
"""

from contextlib import ExitStack

import ml_dtypes
import numpy as np

import concourse.bacc as bacc
import concourse.mybir as mybir
import concourse.tile as tile
from concourse.bass_utils import run_bass_kernel_spmd

B, S, E, H = 4, 1024, 1024, 16
HD = 64
HPC = 8
NPAIR = 4
NET = 8
NTT = 8
P = 128

F32 = mybir.dt.float32
BF16 = mybir.dt.bfloat16
EXP = mybir.ActivationFunctionType.Exp
SCALE = 1.0 / 8.0
BF = ml_dtypes.bfloat16

LAG = 3             # ctx tiles lag behind the exp stream
ETP_BUFS = 16       # e-tile pool depth (bounds max ctx lag)
FILL_PER_TILE = 3.3  # filler matmul units per score tile (208u/64)


def _emit(nc, tc, ctx, aps):
    (wq_d, wk_d, wv_d, xq_d, xk_d, xv_d, wo_d, out_d) = aps

    iw = ctx.enter_context(tc.tile_pool(name="iw", bufs=1))
    ix = ctx.enter_context(tc.tile_pool(name="ix", bufs=1))
    const = ctx.enter_context(tc.tile_pool(name="const", bufs=1))
    etp = ctx.enter_context(tc.tile_pool(name="etp", bufs=ETP_BUFS))
    obp = ctx.enter_context(tc.tile_pool(name="obp", bufs=3))
    rcp = ctx.enter_context(tc.tile_pool(name="rcp", bufs=6))
    pp_sc = ctx.enter_context(tc.tile_pool(name="pp_sc", bufs=2, space="PSUM"))
    pp_ctx = ctx.enter_context(tc.tile_pool(name="pp_ctx", bufs=2, space="PSUM"))
    pp_mm = ctx.enter_context(tc.tile_pool(name="pp_mm", bufs=2, space="PSUM"))

    # split per producer/consumer so dependency tracking stays precise
    qts = {(p, h): const.tile([P, 512], BF16, name=f"qt{p}{h}")
           for p in range(NPAIR) for h in range(2)}
    kts = {(p, h): const.tile([P, 512], BF16, name=f"kt{p}{h}")
           for p in range(NPAIR) for h in range(2)}
    vau = [const.tile([P, 1024], BF16, name=f"vau{tt}") for tt in range(NTT)]
    catt = {(p, sh): const.tile([P, 512], BF16, name=f"cat{p}{sh}")
            for p in range(NPAIR) for sh in range(2)}
    wo_t = const.tile([P, 4096], BF16, name="wo_t")
    warm = const.tile([P, 16], F32, name="warm")

    # ---- input tiles ----
    wqp = [iw.tile([P, 1024], BF16, name=f"wqp{p}") for p in range(NPAIR)]
    wkp = [iw.tile([P, 1024], BF16, name=f"wkp{p}") for p in range(NPAIR)]
    wvt = iw.tile([P, 4096], BF16, name="wvt")
    xqt = [ix.tile([P, 4096], BF16, name=f"xqt{h}") for h in range(2)]
    xkt = [ix.tile([P, 4096], BF16, name=f"xkt{h}") for h in range(2)]
    xvc = [ix.tile([P, 2048], BF16, name=f"xvc{c}") for c in range(4)]

    # ---- input DMAs: two HWDGE rings, chunked, ordered by first use ----
    act_plan = [
        (xqt[0][:, 0:2048], xq_d[:, 0:2048]),
        (xqt[0][:, 2048:4096], xq_d[:, 2048:4096]),
        (wqp[0][:], wq_d[:, 0:1024]),
        (wvt[:, 0:2048], wv_d[:, 0:2048]),
        (wvt[:, 2048:4096], wv_d[:, 2048:4096]),
        (xvc[1][:], xv_d[:, 2048:4096]),
        (xvc[3][:], xv_d[:, 6144:8192]),
        (wqp[1][:], wq_d[:, 1024:2048]),
        (wqp[2][:], wq_d[:, 2048:3072]),
        (wqp[3][:], wq_d[:, 3072:4096]),
    ]
    sp_plan = [
        (xkt[0][:, 0:2048], xk_d[:, 0:2048]),
        (xkt[0][:, 2048:4096], xk_d[:, 2048:4096]),
        (wkp[0][:], wk_d[:, 0:1024]),
        (xkt[1][:, 0:2048], xk_d[:, 4096:6144]),
        (xkt[1][:, 2048:4096], xk_d[:, 6144:8192]),
        (xvc[0][:], xv_d[:, 0:2048]),
        (xvc[2][:], xv_d[:, 4096:6144]),
        (wkp[1][:], wk_d[:, 1024:2048]),
        (xqt[1][:, 0:2048], xq_d[:, 4096:6144]),
        (xqt[1][:, 2048:4096], xq_d[:, 6144:8192]),
        (wkp[2][:], wk_d[:, 2048:3072]),
        (wkp[3][:], wk_d[:, 3072:4096]),
        (wo_t[:, 0:2048], wo_d[:, 0:2048]),
        (wo_t[:, 2048:4096], wo_d[:, 2048:4096]),
    ]
    for dst, src in act_plan:
        nc.scalar.dma_start(out=dst, in_=src)
    # warm the ACT exp table while DMAs stream
    nc.scalar.activation(warm[:, 0:8], warm[:, 8:16], EXP, scale=1.0)
    for dst, src in sp_plan:
        nc.sync.dma_start(out=dst, in_=src)

    # ones blocks of the V augmentation: per tt block [128, 8 heads x 128]
    for tt in range(NTT):
        v4 = vau[tt][:, :].rearrange("p (j q c) -> p j q c", q=2, c=P)
        nc.gpsimd.memset(v4[:, :, 0, HD:P], 1.0)
        nc.gpsimd.memset(v4[:, :, 1, 0:HD], 1.0)

    # ---------------- op emitters ----------------
    proj_done = set()
    fill_units = [0.0]
    open_grp = [0]

    def qk_chunks(which, p, h):
        w = wqp[p] if which == "q" else wkp[p]
        x = xqt[h] if which == "q" else xkt[h]
        dst = qts[(p, h)] if which == "q" else kts[(p, h)]
        state = {}

        def c1():
            state["ps"] = pp_mm.tile([P, 512], F32, name="ps", tag="mm")
            for et in range(4):
                nc.tensor.matmul(
                    state["ps"][:],
                    lhsT=w[:, et * P:(et + 1) * P],
                    rhs=x[:, et * 512:(et + 1) * 512],
                    start=(et == 0), stop=False)

        def c2():
            for et in range(4, NET):
                nc.tensor.matmul(
                    state["ps"][:],
                    lhsT=w[:, et * P:(et + 1) * P],
                    rhs=x[:, et * 512:(et + 1) * 512],
                    start=False, stop=(et == NET - 1))
            nc.vector.tensor_copy(dst[:], state["ps"][:])
            proj_done.add((which, p, h))

        return [(4, c1), (4, c2)]

    vproj_done = set()

    def vproj_chunks(tt):
        x = xvc[tt // 2]
        base = (tt % 2) * 1024
        state = {}

        def c1():
            state["ps"] = pp_mm.tile([P, 512], F32, name="psv", tag="mm")
            for et in range(4):
                nc.tensor.matmul(
                    state["ps"][:],
                    lhsT=x[:, base + et * P:base + (et + 1) * P],
                    rhs=wvt[:, et * 512:(et + 1) * 512],
                    start=(et == 0), stop=False)

        def c2():
            for et in range(4, NET):
                nc.tensor.matmul(
                    state["ps"][:],
                    lhsT=x[:, base + et * P:base + (et + 1) * P],
                    rhs=wvt[:, et * 512:(et + 1) * 512],
                    start=False, stop=(et == NET - 1))
            dstt = vau[tt][:, :].rearrange("p (j q c) -> p j q c", q=2, c=P)
            srcv = state["ps"][:].rearrange("p (j q c) -> p j q c", q=2, c=HD)
            nc.vector.tensor_copy(dstt[:, :, 0, 0:HD], srcv[:, :, 0, :])
            nc.vector.tensor_copy(dstt[:, :, 1, HD:P], srcv[:, :, 1, :])
            vproj_done.add(tt)

        return [(4, c1), (4, c2)]

    norm_count = [0]

    def outgrp_chunk(sti, ih):
        # sti in 0..3 (sh0 s-tiles): contract all 4 pairs' cat columns
        def c():
            ps = pp_mm.tile([P, 512], F32, name="po", tag="mm")
            for p4 in range(NPAIR):
                nc.tensor.matmul(
                    ps[:],
                    lhsT=catt[(p4, 0)][:, sti * P:(sti + 1) * P],
                    rhs=wo_t[:, p4 * 1024 + ih * 512:p4 * 1024 + (ih + 1) * 512],
                    start=(p4 == 0), stop=(p4 == NPAIR - 1))
            ob = obp.tile([P, 512], F32, name="ob", tag="ob")
            nc.vector.tensor_copy(ob[:], ps[:])
            nc.sync.dma_start(
                out=out_d[sti * P:(sti + 1) * P, ih * 512:(ih + 1) * 512],
                in_=ob[:])
        return [(4, c)]

    # attention call order: sh-major. call = sh*4 + p
    CALLS = [(p, 0) for p in range(NPAIR)] + [(p, 1) for p in range(NPAIR)]
    e_tiles = {}

    def sc_exp(i):
        call, tt = divmod(i, NTT)
        p, sh = CALLS[call]
        kth = kts[(p, tt // 4)]
        kcol = (tt % 4) * P
        q = qts[(p, sh)]
        sAB = pp_sc.tile([P, 1024], F32, name="sAB", tag="sc")
        nc.tensor.matmul(sAB[:, 0:512], lhsT=kth[0:HD, kcol:kcol + P],
                         rhs=q[0:HD, :], start=True, stop=True)
        nc.tensor.matmul(sAB[:, 512:1024], lhsT=kth[HD:P, kcol:kcol + P],
                         rhs=q[HD:P, :], start=True, stop=True)
        eAB = etp.tile([P, 1024], BF16, name="eAB", tag="et")
        nc.scalar.activation(eAB[:], sAB[:], EXP, scale=SCALE)
        e_tiles[i] = eAB

    ctx_ps = {}

    def ctx_mm(k):
        call, tt = divmod(k, NTT)
        p, sh = CALLS[call]
        if tt == 0:
            ctx_ps[call] = (
                pp_ctx.tile([P, 512], F32, name="ctxA", tag="ctx"),
                pp_ctx.tile([P, 512], F32, name="ctxB", tag="ctx"),
            )
        ctxA, ctxB = ctx_ps[call]
        eAB = e_tiles.pop(k)
        bA = p * 256
        nc.tensor.matmul(ctxA[:], lhsT=vau[tt][:, bA:bA + P],
                         rhs=eAB[:, 0:512],
                         start=(tt == 0), stop=(tt == NTT - 1))
        nc.tensor.matmul(ctxB[:], lhsT=vau[tt][:, bA + P:bA + 2 * P],
                         rhs=eAB[:, 512:1024],
                         start=(tt == 0), stop=(tt == NTT - 1))
        if tt == NTT - 1:
            normalize(call)

    def normalize(call):
        p, sh = CALLS[call]
        dst = catt[(p, sh)]
        ctxA, ctxB = ctx_ps.pop(call)
        # copy PSUM ctx to SBUF first: frees both ctx banks immediately
        cA = rcp.tile([P, 512], F32, name="cA", tag="rc")
        nc.vector.tensor_copy(cA[:], ctxA[:])
        cB = rcp.tile([P, 512], F32, name="cB", tag="rc")
        nc.vector.tensor_copy(cB[:], ctxB[:])
        # A: ctx rows 0:64, denom rows 64:128 (64 identical copies)
        sA = rcp.tile([P, 512], F32, name="sA", tag="rc")
        rA = rcp.tile([P, 512], F32, name="rA", tag="rc")
        nc.sync.dma_start(out=sA[0:HD, :], in_=cA[HD:P, :])
        nc.vector.reciprocal_approx_fast(rA[0:HD, :], sA[0:HD, :])
        nc.vector.tensor_mul(dst[0:HD, :], cA[0:HD, :], rA[0:HD, :])
        # B: denom rows 0:64, ctx rows 64:128
        rB = rcp.tile([P, 512], F32, name="rB", tag="rc")
        nc.vector.reciprocal_approx_fast(rB[0:HD, :], cB[0:HD, :])
        nc.sync.dma_start(out=rB[HD:P, :], in_=rB[0:HD, :])
        nc.vector.tensor_mul(dst[HD:P, :], cB[HD:P, :], rB[HD:P, :])
        norm_count[0] += 1

    # ---------------- filler machinery ----------------
    def mk(dep, chunks):
        return {"chunks": chunks, "next": 0, "dep": dep}

    always = lambda: True
    fillers = [
        mk(always, qk_chunks("k", 0, 1)),
        mk(always, vproj_chunks(0)),
        mk(always, vproj_chunks(1)),
        mk(always, vproj_chunks(2)),
        mk(always, vproj_chunks(3)),
        mk(always, qk_chunks("k", 1, 0)),
        mk(always, qk_chunks("q", 1, 0)),
        mk(always, vproj_chunks(4)),
        mk(always, vproj_chunks(5)),
        mk(always, vproj_chunks(6)),
        mk(always, vproj_chunks(7)),
        mk(always, qk_chunks("k", 1, 1)),
        mk(always, qk_chunks("k", 2, 0)),
        mk(always, qk_chunks("q", 2, 0)),
        mk(always, qk_chunks("k", 2, 1)),
        mk(always, qk_chunks("q", 0, 1)),
        mk(always, qk_chunks("k", 3, 0)),
        mk(always, qk_chunks("q", 3, 0)),
        mk(always, qk_chunks("k", 3, 1)),
        mk(always, qk_chunks("q", 1, 1)),
        mk(always, qk_chunks("q", 2, 1)),
        mk(always, qk_chunks("q", 3, 1)),
    ]
    qk_order = [("k", 0, 1), None, None, None, None, ("k", 1, 0), ("q", 1, 0),
                None, None, None, None, ("k", 1, 1), ("k", 2, 0), ("q", 2, 0),
                ("k", 2, 1), ("q", 0, 1), ("k", 3, 0), ("q", 3, 0),
                ("k", 3, 1), ("q", 1, 1), ("q", 2, 1), ("q", 3, 1)]
    qk_items = {key: it for it, key in zip(fillers, qk_order) if key}
    vp_order = [None, 0, 1, 2, 3, None, None, 4, 5, 6, 7]
    vp_items = {key: it for it, key in zip(fillers, vp_order)
                if key is not None}
    for sti in range(4):
        for ih in range(2):
            fillers.append(mk(lambda: norm_count[0] >= 4,
                              outgrp_chunk(sti, ih)))

    def emit_item_chunk(it):
        n = it["next"]
        if n == 0:
            open_grp[0] += 1
        u, fn = it["chunks"][n]
        fn()
        it["next"] += 1
        if it["next"] >= len(it["chunks"]):
            open_grp[0] -= 1
        fill_units[0] += u
        return u

    def emit_filler_chunk():
        for it in fillers:
            n = it["next"]
            if n >= len(it["chunks"]):
                continue
            if n == 0 and (open_grp[0] >= 2 or not it["dep"]()):
                continue
            return emit_item_chunk(it)
        return 0

    def finish_item(it):
        # a force-start while 2 groups are open would clobber a PSUM slot:
        # close the open groups first
        if it["next"] == 0 and open_grp[0] >= 2:
            for other in fillers:
                if other is not it and 0 < other["next"] < len(other["chunks"]):
                    while other["next"] < len(other["chunks"]):
                        emit_item_chunk(other)
        while it["next"] < len(it["chunks"]):
            emit_item_chunk(it)

    def ensure_proj(which, p, h):
        if (which, p, h) not in proj_done:
            finish_item(qk_items[(which, p, h)])

    def ensure_vproj(tt):
        if tt not in vproj_done:
            finish_item(vp_items[tt])

    # ---------------- main schedule ----------------
    for u, fn in qk_chunks("k", 0, 0):
        fn()
    for u, fn in qk_chunks("q", 0, 0):
        fn()
    proj_done.add(("k", 0, 0))
    proj_done.add(("q", 0, 0))

    ctx_next = [0]

    def emit_ctx_upto(limit):
        while ctx_next[0] <= limit:
            k = ctx_next[0]
            _, tt = divmod(k, NTT)
            ensure_vproj(tt)
            ctx_mm(k)
            ctx_next[0] += 1

    for i in range(64):
        call, tt = divmod(i, NTT)
        p, sh = CALLS[call]
        emit_ctx_upto(i - ETP_BUFS)
        ensure_proj("q", p, sh)
        ensure_proj("k", p, 0)
        if tt >= 4:
            ensure_proj("k", p, 1)
        sc_exp(i)
        if i >= LAG:
            emit_ctx_upto(i - LAG)
        while fill_units[0] < (i + 1) * FILL_PER_TILE:
            if emit_filler_chunk() == 0:
                break

    emit_ctx_upto(63)
    for it in fillers:
        finish_item(it)

    # ---- tail: st4..7 output projections ----
    # pre-accumulate pairs 0..2 in freed PSUM banks, finish with pair 3
    # right after the final normalize.
    tg4 = pp_sc.tile([P, 1024], F32, name="tg4", tag="sc")
    tg5 = pp_sc.tile([P, 1024], F32, name="tg5", tag="sc")
    t6 = (pp_ctx.tile([P, 512], F32, name="t6a", tag="ctx"),
          pp_ctx.tile([P, 512], F32, name="t6b", tag="ctx"))
    t7 = (pp_mm.tile([P, 512], F32, name="t7a", tag="mm"),
          pp_mm.tile([P, 512], F32, name="t7b", tag="mm"))

    def tail_mm(ps2, sti, p4, start, stop):
        for ih in range(2):
            dst = ps2[ih][:] if isinstance(ps2, tuple) else \
                ps2[:, ih * 512:(ih + 1) * 512]
            nc.tensor.matmul(
                dst,
                lhsT=catt[(p4, 1)][:, (sti - 4) * P:(sti - 3) * P],
                rhs=wo_t[:, p4 * 1024 + ih * 512:p4 * 1024 + (ih + 1) * 512],
                start=start, stop=stop)

    groups = [(4, tg4), (5, tg5), (6, t6), (7, t7)]
    for p4 in range(3):
        for sti, ps2 in groups:
            tail_mm(ps2, sti, p4, start=(p4 == 0), stop=False)
    for sti, ps2 in groups:
        tail_mm(ps2, sti, 3, start=False, stop=True)
        ob = obp.tile([P, 1024], F32, name="obt", tag="obt")
        if isinstance(ps2, tuple):
            nc.vector.tensor_copy(ob[:, 0:512], ps2[0][:])
            nc.vector.tensor_copy(ob[:, 512:1024], ps2[1][:])
        else:
            nc.vector.tensor_copy(ob[:], ps2[:])
        nc.sync.dma_start(out=out_d[sti * P:(sti + 1) * P, :], in_=ob[:])


_CACHE = {}


def build():
    if "nc" in _CACHE:
        return _CACHE["nc"]
    nc = bacc.Bacc("TRN2", target_bir_lowering=False, debug=False)
    wq_d = nc.dram_tensor("wq", [P, 4096], BF16, kind="ExternalInput").ap()
    wk_d = nc.dram_tensor("wk", [P, 4096], BF16, kind="ExternalInput").ap()
    wv_d = nc.dram_tensor("wv", [P, 4096], BF16, kind="ExternalInput").ap()
    xq_d = nc.dram_tensor("xq", [P, 8192], BF16, kind="ExternalInput").ap()
    xk_d = nc.dram_tensor("xk", [P, 8192], BF16, kind="ExternalInput").ap()
    xv_d = nc.dram_tensor("xv", [P, 8192], BF16, kind="ExternalInput").ap()
    wo_d = nc.dram_tensor("wo", [P, 4096], BF16, kind="ExternalInput").ap()
    out_d = nc.dram_tensor("out", [S, E], F32, kind="ExternalOutput").ap()
    with tile.TileContext(nc) as tc, ExitStack() as ctx:
        _emit(nc, tc, ctx, (wq_d, wk_d, wv_d, xq_d, xk_d, xv_d, wo_d, out_d))
    nc.compile()
    _CACHE["nc"] = nc
    return nc


def make_in_maps(query, key, value, Wq, Wk, Wv, Wo):
    in_maps = []
    for c in range(8):
        b, g = divmod(c, 2)
        hs = slice(g * HPC, (g + 1) * HPC)

        def bf(a):
            return np.ascontiguousarray(a, dtype=np.float32).astype(BF)

        # per-pair Q/K weights: w[ep, p*1024 + et*128 + hp*64 + d]
        def wqk(Wfull):
            A = np.asarray(Wfull[hs], np.float32)          # [8, 64, 1024]
            Ap = A.reshape(NPAIR, 2, HD, NET, P)           # [p, hp, d, et, ep]
            Ap = Ap.transpose(4, 0, 3, 1, 2)               # [ep, p, et, hp, d]
            return bf(Ap.reshape(P, 4096))

        # x for Q/K: x[ep, h*4096 + et*512 + s]
        def xqk(x):
            X = np.asarray(x, np.float32)                  # [1024 s, 1024 e]
            Xp = X.reshape(2, 512, NET, P)                 # [h, s, et, ep]
            Xp = Xp.transpose(3, 0, 2, 1)                  # [ep, h, et, s]
            return bf(Xp.reshape(P, 8192))

        # x for V: x[ep, tt*1024 + et*128 + tl]
        V = np.asarray(value[b], np.float32)
        Vp = V.reshape(NTT, P, NET, P).transpose(3, 0, 2, 1)
        xv = bf(Vp.reshape(P, 8192))

        # wv: [ep, et*512 + h*64 + d]
        Bv = np.asarray(Wv[hs], np.float32)
        Bp = Bv.reshape(HPC, HD, NET, P).transpose(3, 2, 0, 1)
        wv = bf(Bp.reshape(P, 4096))

        # wo: [f, p4*1024 + i]
        C = np.asarray(Wo[:, g * 512:(g + 1) * 512], np.float32)
        Cp = C.reshape(E, NPAIR, P).transpose(2, 1, 0)
        wo = bf(Cp.reshape(P, 4096))

        in_maps.append({
            "wq": wqk(Wq), "wk": wqk(Wk), "wv": wv,
            "xq": xqk(query[b]), "xk": xqk(key[b]), "xv": xv,
            "wo": wo,
        })
    return in_maps


def kernel(query, key, value, Wq, Wk, Wv, Wo):
    nc = build()
    in_maps = make_in_maps(query, key, value, Wq, Wk, Wv, Wo)
    res = run_bass_kernel_spmd(nc, in_maps, list(range(8))).results
    out = np.empty((B, S, E), np.float32)
    for b in range(B):
        out[b] = res[2 * b]["out"] + res[2 * b + 1]["out"]
    return out
